# revision 27
# baseline (speedup 1.0000x reference)
# Trainium2 Bass kernel for nn_Invo2D (involution-style dynamic conv).
#
# Math (verified vs reference):
#   t1 = x @ W1 + b1                      [pix, 64]
#   t2 = t1 @ W2 + b2                     [pix, 144] = [g:16, j:9]
#   P[pix, f] = 3x3 SAME patches, f = tap*256 + ch   (tap row-major)
#   out[pix, co] = sum_j t2[pix, 9*(co//16)+j] * P[pix, 9*co+j]
#
# Sharding: data-parallel over batch, 1 image per NeuronCore (8 cores).
#
# Per-core layout: partition p = wq*64 + h (wq = w//32); per-partition free dim
# holds 34 w-slots of 256 channels (slot s <-> w = 32*wq + s - 1, edge slots
# zero).  Spatial taps become free-dim offsets (w) plus row-shifted copies of
# x (h +- 1), so the data-dependent multiply-reduce runs lane-local on the
# Vector engine.
#
# Host pre-arranges x in bf16 (slot images x0/xu/xd + channel-major xcm) so
# the device program per wl-chunk is: DMA quarters in, 4 matmuls/tile for t2,
# elementwise products into a 16-padded slot buffer, a fold tree, bf16 out
# DMA (host upcasts).  Chunk 0 is split in half so products start after only
# 4 tiles.  Within a ki row the 3 w-taps are adjacent channels, so product
# pieces span 256-boundaries and only split at f = 768k.

import os
import numpy as np
import ml_dtypes

H, W, C = 64, 64, 256
M144, D = 144, 64
NCORES = 8
SLOTS = 34            # w slots per partition: slot s <-> w = 32*wq + s - 1
XF = SLOTS * C        # 8704 x-elems per partition
WLC = 8               # wl chunk size
NCHUNK = 32 // WLC    # 4 chunks
M16F = WLC * 4096     # product-chunk free size (16-padded slots per co)
SOFF = 4              # product slot offset: tap j -> slot j+4 (slots 4..12)
POOL_ELEMS = int(os.environ.get("POOL_ELEMS", "0"))  # per-chunk free elems on Pool

_cache = {}


def _rect_decomp(r0, r1):
    """[r0, r1) in (gc, j) space (gc = r//9, j = r%9) -> rects (gc0, ngc, j0, nj)."""
    out = []
    gc0, j0 = divmod(r0, 9)
    if j0 != 0:
        end = min(r1, (gc0 + 1) * 9)
        out.append((gc0, 1, j0, end - r0))
        r0 = end
        if r0 == r1:
            return out
        gc0, j0 = divmod(r0, 9)
    nfull = (r1 - r0) // 9
    if nfull:
        out.append((gc0, nfull, 0, 9))
        r0 += nfull * 9
        gc0 += nfull
    if r0 < r1:
        out.append((gc0, 1, 0, r1 - r0))
    return out


def _build_pieces():
    """Mult pieces: (g, gc0, ngc, j0, nj, tap).

    Within one ki row (3 horizontal taps), x addressing is linear in f: the
    w-taps are adjacent 256-channel columns, so one op can span 256-boundaries
    (the AP walks into the next w column, which is exactly the next tap's
    data).  Pieces therefore split only at ki-row changes (f = 768k), giving
    one full 16x9 rect per group except g=5 and g=10."""
    pieces = []
    for g in range(16):
        f0, f1 = 144 * g, 144 * g + 144
        kb = None
        for k in (768, 1536):
            if f0 < k < f1:
                kb = k
        if kb is None:
            pieces.append((g, 0, 16, 0, 9, f0 // 256))
        else:
            for (a, b) in ((f0, kb), (kb, f1)):
                for (gc0, ngc, j0, nj) in _rect_decomp(a - f0, b - f0):
                    tap = (f0 + 9 * gc0 + j0) // 256
                    pieces.append((g, gc0, ngc, j0, nj, tap))
    return pieces


def _split_pieces(pieces):
    """Assign the largest pieces to the Pool engine up to POOL_ELEMS/wl-chunk."""
    order = sorted(range(len(pieces)),
                   key=lambda i: -(pieces[i][2] * pieces[i][4]))
    pool_idx = set()
    budget = POOL_ELEMS // WLC
    for i in order:
        sz = pieces[i][2] * pieces[i][4]
        if sz <= budget:
            pool_idx.add(i)
            budget -= sz
        if budget <= 0:
            break
    dve = [p for i, p in enumerate(pieces) if i not in pool_idx]
    pool = [p for i, p in enumerate(pieces) if i in pool_idx]
    return dve, pool


def _build_program():
    import concourse.bass as bass
    import concourse.tile as tile
    from concourse import bacc, mybir
    from concourse.masks import make_identity

    f32 = mybir.dt.float32
    bf16 = mybir.dt.bfloat16
    AP = bass.AP

    nc = bacc.Bacc(None, target_bir_lowering=False)
    x0_d = nc.dram_tensor("x0", [128, XF], bf16, kind="ExternalInput")
    xu_d = nc.dram_tensor("xu", [128, XF], bf16, kind="ExternalInput")
    xd_d = nc.dram_tensor("xd", [128, XF], bf16, kind="ExternalInput")
    xcm_d = nc.dram_tensor("xcm", [128, 8192], bf16, kind="ExternalInput")
    w1_d = nc.dram_tensor("w1", [2, 128, D], bf16, kind="ExternalInput")
    b1_d = nc.dram_tensor("b1row", [1, D], bf16, kind="ExternalInput")
    w2_d = nc.dram_tensor("w2aug", [D + 1, M144], bf16, kind="ExternalInput")
    out_d = nc.dram_tensor("out", [H, W, C], bf16, kind="ExternalOutput")

    DVE_PIECES, POOL_PIECES = _split_pieces(_build_pieces())

    with tile.TileContext(nc) as tc:
        with (
            tc.tile_pool(name="singles", bufs=1) as singles,
            tc.tile_pool(name="xbufs", bufs=1) as xbufs,
            tc.tile_pool(name="big", bufs=1) as big,
            tc.tile_pool(name="folds", bufs=1) as folds,
            tc.tile_pool(name="outp", bufs=2) as outp,
            tc.tile_pool(name="ps1", bufs=2, space="PSUM") as ps1p,
            tc.tile_pool(name="ps2", bufs=2, space="PSUM") as ps2p,
            tc.tile_pool(name="psf", bufs=4, space="PSUM") as psfp,
        ):
            # ---- constants ----
            w1a = singles.tile([128, D], bf16)
            w1b = singles.tile([128, D], bf16)
            nc.sync.dma_start(out=w1a[:], in_=w1_d[0])
            nc.sync.dma_start(out=w1b[:], in_=w1_d[1])
            b1sb = singles.tile([1, D], bf16)
            nc.sync.dma_start(out=b1sb[:], in_=b1_d[:])
            w2sb = singles.tile([D + 1, M144], bf16)
            nc.sync.dma_start(out=w2sb[:], in_=w2_d[:])
            ones1 = singles.tile([1, 128], bf16)
            nc.gpsimd.memset(ones1[:], 1.0)
            ident = singles.tile([128, 128], bf16)
            make_identity(nc, ident[:])

            # persistent t1 buffer (8 tile-columns) with bias row (1.0)
            t1tbig = singles.tile([D + 1, 1024], bf16)
            nc.gpsimd.memset(t1tbig[D:D + 1, :], 1.0)

            X0 = xbufs.tile([128, XF], bf16)
            XU = xbufs.tile([128, XF], bf16)   # row h+1
            XD = xbufs.tile([128, XF], bf16)   # row h-1
            xcm = singles.tile([128, 8192], bf16)
            W16c = [big.tile([128, WLC * 256], bf16, name=f"w16_{i}",
                             tag=f"w16_{i}") for i in range(3)]
            M16 = big.tile([128, M16F], bf16, tag="m16")
            F1bs = [folds.tile([128, WLC * 1024], bf16, name=f"f1b_{i}")
                    for i in range(2)]

            # slot quarters: Q[c] = slots [lo, hi)
            QUART = [(0, 10), (10, 18), (18, 26), (26, 34)]
            XBUF = {-1: XD, 0: X0, 1: XU}

            def emit_tiles(w16, t0, ntile):
                """t1/t2 for tiles t0..t0+ntile, scatter into w16 cols 0..ntile."""
                for tt in range(ntile):
                    t = t0 + tt
                    ps1 = ps1p.tile([D, 128], f32)
                    xc0 = AP(xcm.tensor, (0 * 32 + t) * 128,
                             [[8192, 128], [1, 128]])
                    xc1 = AP(xcm.tensor, (1 * 32 + t) * 128,
                             [[8192, 128], [1, 128]])
                    nc.tensor.matmul(ps1[:], lhsT=w1a[:], rhs=xc0,
                                     start=True, stop=False)
                    nc.tensor.matmul(ps1[:], lhsT=w1b[:], rhs=xc1,
                                     start=False, stop=False)
                    nc.tensor.matmul(ps1[:], lhsT=b1sb[:], rhs=ones1[:],
                                     start=False, stop=True)
                    tcol = t1tbig[0:D, t % 8 * 128:(t % 8 + 1) * 128]
                    nc.scalar.copy(out=tcol, in_=ps1[:])
                    ps2 = ps2p.tile([128, M144], f32)
                    lhsT = AP(t1tbig.tensor, t % 8 * 128,
                              [[1024, D + 1], [1, 128]])
                    nc.tensor.matmul(ps2[:], lhsT=lhsT, rhs=w2sb[:],
                                     start=True, stop=True)
                    # scatter t2[m=9g+j] into slots [col, 16g + SOFF + j]
                    nc.scalar.copy(
                        out=AP(w16.tensor, tt * 256 + SOFF,
                               [[WLC * 256, 128], [16, 16], [1, 9]]),
                        in_=AP(ps2.tensor, 0, [[M144, 128], [9, 16], [1, 9]]),
                    )

            def emit_products(w16, wl_abs, wl_lo, nw):
                """Products into M16 slots 4..12 for wl [wl_lo, wl_lo+nw)."""
                def emit_piece(eng, g, gc0, ngc, j0, nj, tap):
                    di, dj = tap // 3 - 1, tap % 3 - 1
                    xb = XBUF[di]
                    ch0 = 144 * g + 9 * gc0 + j0 - 256 * tap
                    in0 = AP(xb.tensor, (wl_abs + dj + 1) * 256 + ch0,
                             [[XF, 128], [256, nw], [9, ngc], [1, nj]])
                    in1 = AP(w16.tensor, 16 * g + SOFF + j0,
                             [[WLC * 256, 128], [256, nw], [0, ngc], [1, nj]])
                    o = AP(M16.tensor,
                           wl_lo * 4096 + (16 * g + gc0) * 16 + SOFF + j0,
                           [[M16F, 128], [4096, nw], [16, ngc], [1, nj]])
                    eng.tensor_mul(o, in0, in1)

                for pc in POOL_PIECES:
                    emit_piece(nc.gpsimd, *pc)
                for pc in DVE_PIECES:
                    emit_piece(nc.vector, *pc)

            for c in range(NCHUNK):
                lo, hi = QUART[c]
                QF = (hi - lo) * 256
                nc.scalar.dma_start(
                    out=AP(xcm.tensor, c * 8 * 128,
                           [[8192, 128], [4096, 2], [1, 1024]]),
                    in_=AP(xcm_d, c * 8 * 128,
                           [[8192, 128], [4096, 2], [1, 1024]]))
                nc.sync.dma_start(
                    out=AP(X0.tensor, lo * 256, [[XF, 128], [1, QF]]),
                    in_=AP(x0_d, lo * 256, [[XF, 128], [1, QF]]))
                nc.sync.dma_start(
                    out=AP(XU.tensor, lo * 256, [[XF, 128], [1, QF]]),
                    in_=AP(xu_d, lo * 256, [[XF, 128], [1, QF]]))
                nc.sync.dma_start(
                    out=AP(XD.tensor, lo * 256, [[XF, 128], [1, QF]]),
                    in_=AP(xd_d, lo * 256, [[XF, 128], [1, QF]]))

                if c == 0:
                    # split first chunk so products start after only 2 tiles
                    emit_tiles(W16c[0], 0, 2)
                    emit_products(W16c[0], 0, 0, 2)
                    emit_tiles(W16c[1], 2, 6)
                    emit_products(W16c[1], 2, 2, 6)
                else:
                    w16 = W16c[2 if c == 1 else c % 2]
                    emit_tiles(w16, c * 8, 8)
                    emit_products(w16, c * 8, 0, 8)

                # ---- fold tree ----
                # F1: slots (4..7) + (8..11) -> F1b[wl*1024 + 4*co + s]
                F1b = F1bs[c % 2]
                nc.vector.tensor_add(
                    AP(F1b.tensor, 0, [[WLC * 1024, 128], [1024, WLC], [4, 256], [1, 4]]),
                    AP(M16.tensor, SOFF, [[M16F, 128], [4096, WLC], [16, 256], [1, 4]]),
                    AP(M16.tensor, SOFF + 4, [[M16F, 128], [4096, WLC], [16, 256], [1, 4]]),
                )
                # F2 + F3 + final on PE: psum = I@M16[slot12]
                # + sum_s I@F1b[s]; Act casts to bf16
                outc = outp.tile([128, WLC * 256], bf16)
                for q in range(4):
                    pq = psfp.tile([128, 512], f32)
                    nc.tensor.matmul(
                        pq[:], lhsT=ident[:],
                        rhs=AP(M16.tensor, 2 * q * 4096 + SOFF + 8,
                               [[M16F, 128], [4096, 2], [16, 256]]),
                        start=True, stop=False)
                    for s in range(4):
                        nc.tensor.matmul(
                            pq[:], lhsT=ident[:],
                            rhs=AP(F1b.tensor, 2 * q * 1024 + s,
                                   [[WLC * 1024, 128], [1024, 2], [4, 256]]),
                            start=False, stop=(s == 3))
                    nc.scalar.copy(
                        out=AP(outc.tensor, q * 512,
                               [[WLC * 256, 128], [1, 512]]),
                        in_=pq[:])
                wl0 = c * WLC
                nc.sync.dma_start(
                    out=AP(out_d, wl0 * 256, [[W * C, 64], [1, WLC * 256]]),
                    in_=AP(outc.tensor, 0, [[WLC * 256, 64], [1, WLC * 256]]),
                )
                nc.sync.dma_start(
                    out=AP(out_d, (32 + wl0) * 256, [[W * C, 64], [1, WLC * 256]]),
                    in_=AP(outc.tensor, 64 * WLC * 256,
                           [[WLC * 256, 64], [1, WLC * 256]]),
                )
    nc.compile()
    return nc


def _get_program():
    if "nc" not in _cache:
        _cache["nc"] = _build_program()
    return _cache["nc"]


def _host_x(x_img):
    """x_img [H, W, C] f32 -> (x0, xu, xd, xcm) bf16 device images."""
    bf = ml_dtypes.bfloat16
    xb = x_img.astype(bf)

    def slot_img(src):
        img = np.zeros((128, SLOTS, C), dtype=bf)
        img[0:64, 1:34, :] = src[:, 0:33, :]      # wq0: slot s <-> w = s-1
        img[64:128, 0:33, :] = src[:, 31:64, :]   # wq1: slot s <-> w = 31+s
        return img.reshape(128, XF)

    zr = np.zeros((1, W, C), dtype=bf)
    x0 = slot_img(xb)
    xu = slot_img(np.concatenate([xb[1:], zr], axis=0))    # row h+1
    xd = slot_img(np.concatenate([zr, xb[:-1]], axis=0))   # row h-1
    # xcm[ch, (half*32+t)*128 + wq*64 + h] = x[h, 32wq+t, 128half+ch]
    a = xb.transpose(2, 1, 0)                  # [c, w, h]
    a = a.reshape(2, 128, 2, 32, H)            # [half, ch, wq, t, h]
    a = a.transpose(1, 0, 3, 2, 4)             # [ch, half, t, wq, h]
    xcm = np.ascontiguousarray(a.reshape(128, 8192))
    return x0, xu, xd, xcm


def kernel(x, W1, b1, W2, b2, trace=False):
    from concourse.bass_utils import run_bass_kernel_spmd

    nc = _get_program()
    bf = ml_dtypes.bfloat16
    w1_h = np.ascontiguousarray(W1.astype(bf).reshape(2, 128, D))
    b1_h = np.ascontiguousarray(b1.astype(bf).reshape(1, D))
    w2_h = np.ascontiguousarray(
        np.concatenate([W2, b2[None, :]], axis=0).astype(bf))
    in_maps = []
    for i in range(NCORES):
        x0, xu, xd, xcm = _host_x(np.asarray(x[i], dtype=np.float32))
        in_maps.append({
            "x0": x0, "xu": xu, "xd": xd, "xcm": xcm,
            "w1": w1_h, "b1row": b1_h, "w2aug": w2_h,
        })
    res = run_bass_kernel_spmd(nc, in_maps, core_ids=list(range(NCORES)),
                               trace=trace)
    out = np.stack([res.results[i]["out"] for i in range(NCORES)], axis=0)
    out = out.astype(np.float32)
    if trace:
        return out, res
    return out
